# revision 5
# baseline (speedup 1.0000x reference)
"""TFN-style convolution kernel for 8 trn2 NeuronCores.

Reference computation (B=4, N=512, RBF=32, F=16):
  r00 = image @ W00 + b00                        (B,N,N,F)
  out1[m,a,f]   = sum_b r00[m,a,b,f] * f0[m,b,f]
  out2[m,a,f,i] = sum_b v[m,a,b,i] r01[m,a,b,f] f0[m,b,f]
  out3[m,a,f,i] = sum_b r10[m,a,b,f] feat1[m,b,f,i]
  out4[m,a,f]   = sum_b,j v_j r11 feat1_j
  out5[m,a,f,i] = eps_ijk sum_b v_j r11 feat1_k

Strategy: shard (m, a-half) over 8 cores -- fully data-parallel, no
collectives.  Fold the per-b feature factors into the matmul weights:
  M00[b,r,f] = W00[r,f]*f0[b,f]   -> out1 = sum_{b,r} image[a,(b,r)] M00
and analogously M01/M10/M11k.  The v-coupled terms use img2_j = v_j*image
(elementwise, VectorE) as the moving operand.  The contraction (b,r) is
tiled b-major: K-chunk = 128 consecutive b at a single r, so one compact
v tile [128b, 256a] serves all 33 r-chunks (no broadcast materialization).
r=33 channels: channel 32 is a constant ones-plane which folds the radial
biases in exactly.  All inputs bf16 (host cast), accumulate fp32 in PSUM.
"""

import numpy as np

import concourse.bass as bass
import concourse.bacc as bacc
import concourse.mybir as mybir
from concourse.tile import TileContext
from concourse.bass_utils import run_bass_kernel_spmd

B, N, RBF, F = 4, 512, 32, 16
NCORES = 8
A = 256            # output rows 'a' per core
NB = 4             # b blocks
PB = 128           # b per block (partition dim)
R = RBF + 1        # radial channels + ones plane (bias)
BF16 = mybir.dt.bfloat16
F32 = mybir.dt.float32
NP_BF16 = mybir.dt.np(BF16)

OUT_ROWS = 176     # 16 out1 + 48 out3 + 3*16 out2 + 16 out4 + 3*16 out5


def _build_graph():
    nc = bacc.Bacc(None)
    img_t = nc.declare_dram_parameter("img_t", [NB, PB, R, A], BF16, isOutput=False)
    m_a = nc.declare_dram_parameter("m_a", [PB, NB, R, 64], BF16, isOutput=False)
    m_b = nc.declare_dram_parameter("m_b", [PB, NB, R, 64], BF16, isOutput=False)
    v_t = nc.declare_dram_parameter("v_t", [PB, NB, 3, A], BF16, isOutput=False)
    out = nc.declare_dram_parameter("out", [OUT_ROWS, A], F32, isOutput=True)

    with TileContext(nc) as tc:
        with (
            tc.tile_pool(name="const", bufs=1) as cpool,
            tc.tile_pool(name="img", bufs=2) as ipool,
            tc.tile_pool(name="img2", bufs=2) as i2pool,
            tc.tile_pool(name="psum", bufs=1, space="PSUM") as ppool,
            tc.tile_pool(name="outp", bufs=1) as opool,
        ):
            ma_sb = cpool.tile([PB, NB * R * 64], BF16, name="ma_sb")
            mb_sb = cpool.tile([PB, NB * R * 64], BF16, name="mb_sb")
            vt_sb = cpool.tile([PB, NB * 3 * A], BF16, name="vt_sb")
            nc.sync.dma_start(ma_sb[:], m_a[:].rearrange("p b r c -> p (b r c)"))
            nc.sync.dma_start(mb_sb[:], m_b[:].rearrange("p b r c -> p (b r c)"))
            nc.sync.dma_start(vt_sb[:], v_t[:].rearrange("p b j a -> p (b j a)"))

            psA = ppool.tile([64, A], F32, name="psA")
            psB = [ppool.tile([64, A], F32, name=f"psB{j}") for j in range(3)]

            for blk in range(NB):
                img_sb = ipool.tile([PB, R * A], BF16, tag="img")
                nc.sync.dma_start(
                    img_sb[:], img_t[blk].rearrange("p r a -> p (r a)")
                )
                img_v = img_sb[:].rearrange("p (r a) -> p r a", a=A)

                # img2_j = image * v_j, broadcast v along r (stride-0 mid dim)
                img2 = []
                for j in range(3):
                    t = i2pool.tile([PB, R * A], BF16, tag=f"img2_{j}")
                    vs = vt_sb[:, (blk * 3 + j) * A : (blk * 3 + j + 1) * A]
                    v_bcast = bass.AP(
                        vs.tensor, vs.offset, [vs.ap[0], [0, R], vs.ap[-1]]
                    )
                    nc.vector.tensor_tensor(
                        out=t[:].rearrange("p (r a) -> p r a", a=A),
                        in0=img_v,
                        in1=v_bcast,
                        op=mybir.AluOpType.mult,
                    )
                    img2.append(t)

                for r in range(R):
                    start = blk == 0 and r == 0
                    stop = blk == NB - 1 and r == R - 1
                    w_off = (blk * R + r) * 64
                    nc.tensor.matmul(
                        psA[:],
                        ma_sb[:, w_off : w_off + 64],
                        img_sb[:, r * A : (r + 1) * A],
                        start=start,
                        stop=stop,
                    )
                    for j in range(3):
                        nc.tensor.matmul(
                            psB[j][:],
                            mb_sb[:, w_off : w_off + 64],
                            img2[j][:, r * A : (r + 1) * A],
                            start=start,
                            stop=stop,
                        )

            # Epilogue. DVE ops require src/dst to share the start partition,
            # so: copy PSUM tiles out whole, realign T_jk slices onto
            # partitions 0-15 with small SBUF->SBUF DMAs, combine there.
            # psA rows: 0:16 out1, 16:64 out3[(f,i)]
            # psB[j] rows: 0:16 out2_j, 16+16k+f = T[j,k,f]
            sbA = opool.tile([64, A], F32, name="sbA")
            nc.vector.tensor_copy(out=sbA[:], in_=psA[:])
            scrf = []
            for j in range(3):
                t = opool.tile([64, A], F32, name=f"scrf{j}")
                nc.vector.tensor_copy(out=t[:], in_=psB[j][:])
                scrf.append(t)
            alg = []
            for j in range(3):
                t = opool.tile([16, 3 * A], F32, name=f"alg{j}")
                for k in range(3):
                    nc.sync.dma_start(
                        t[:, k * A : (k + 1) * A],
                        scrf[j][16 + 16 * k : 32 + 16 * k, :],
                    )
                alg.append(t)

            add = mybir.AluOpType.add
            sub = mybir.AluOpType.subtract
            sb45 = opool.tile([16, 4 * A], F32, name="sb45")
            # out4 = T00 + T11 + T22
            nc.vector.tensor_tensor(
                out=sb45[:, 0:A], in0=alg[0][:, 0:A], in1=alg[1][:, A : 2 * A], op=add
            )
            nc.vector.tensor_tensor(
                out=sb45[:, 0:A], in0=sb45[:, 0:A], in1=alg[2][:, 2 * A : 3 * A], op=add
            )
            # out5_0 = T12 - T21 ; out5_1 = T20 - T02 ; out5_2 = T01 - T10
            nc.vector.tensor_tensor(
                out=sb45[:, A : 2 * A],
                in0=alg[1][:, 2 * A : 3 * A],
                in1=alg[2][:, A : 2 * A],
                op=sub,
            )
            nc.vector.tensor_tensor(
                out=sb45[:, 2 * A : 3 * A],
                in0=alg[2][:, 0:A],
                in1=alg[0][:, 2 * A : 3 * A],
                op=sub,
            )
            nc.vector.tensor_tensor(
                out=sb45[:, 3 * A : 4 * A],
                in0=alg[0][:, A : 2 * A],
                in1=alg[1][:, 0:A],
                op=sub,
            )

            nc.sync.dma_start(out[0:64, :], sbA[:])
            for j in range(3):
                nc.sync.dma_start(
                    out[64 + 16 * j : 80 + 16 * j, :], scrf[j][0:16, :]
                )
            nc.sync.dma_start(
                out[112:176, :].rearrange("(s f) a -> f s a", s=4),
                sb45[:].rearrange("p (s a) -> p s a", s=4),
            )

    nc.compile()
    return nc


_NC_CACHE = None


def _get_graph():
    global _NC_CACHE
    if _NC_CACHE is None:
        _NC_CACHE = _build_graph()
    return _NC_CACHE


def _prep_core(image, vectors, feat0, feat1, Ws, bs, m, a0):
    """Host-side shard + layout prep for one core (numpy, fp32 -> bf16)."""
    W00, W01, W10, W11 = Ws
    b00, b01, b10, b11 = bs
    h0 = feat0[m, :, :, 0].astype(np.float32)          # (N, F)
    f1 = feat1[m].astype(np.float32)                   # (N, F, 3)

    # image transposed (b, r, a) + ones plane -> [NB, PB, R, A]
    img = image[m, a0 : a0 + A]                        # (A, N, RBF)
    img3 = np.empty((N, R, A), np.float32)
    img3[:, :RBF, :] = np.transpose(img, (1, 2, 0))
    img3[:, RBF, :] = 1.0
    img_t = np.ascontiguousarray(
        img3.reshape(NB, PB, R, A), dtype=NP_BF16
    )

    # v transposed (b, j, a) -> [PB, NB, 3, A]
    v = vectors[m, a0 : a0 + A]                        # (A, N, 3)
    v3 = np.transpose(v, (1, 2, 0)).reshape(NB, PB, 3, A)
    v_t = np.ascontiguousarray(np.transpose(v3, (1, 0, 2, 3)), dtype=NP_BF16)

    # weight matrices with bias row appended (r = RBF is the ones plane)
    def aug(W, b):
        return np.vstack([W.astype(np.float32), b.astype(np.float32)[None, :]])

    W00a, W01a, W10a, W11a = aug(W00, b00), aug(W01, b01), aug(W10, b10), aug(W11, b11)

    m_a = np.empty((N, R, 64), np.float32)
    m_a[:, :, 0:16] = np.einsum("rf,bf->brf", W00a, h0)
    m_a[:, :, 16:64] = np.einsum("rf,bfi->brfi", W10a, f1).reshape(N, R, 48)
    m_b = np.empty((N, R, 64), np.float32)
    m_b[:, :, 0:16] = np.einsum("rf,bf->brf", W01a, h0)
    # cols 16 + 16k + f  (k-major)
    m_b[:, :, 16:64] = np.einsum("rf,bfk->bkrf", W11a, f1).transpose(0, 2, 1, 3).reshape(N, R, 48)

    def to_param(m_full):  # (N, R, 64) -> [PB, NB, R, 64]
        return np.ascontiguousarray(
            m_full.reshape(NB, PB, R, 64).transpose(1, 0, 2, 3), dtype=NP_BF16
        )

    return dict(img_t=img_t, m_a=to_param(m_a), m_b=to_param(m_b), v_t=v_t)


_LAST_RESULTS = None


def kernel(image, vectors, feat0, feat1, W00, b00, W01, b01, W10, b10, W11, b11):
    global _LAST_RESULTS
    image = np.asarray(image, np.float32)
    vectors = np.asarray(vectors, np.float32)
    feat0 = np.asarray(feat0, np.float32)
    feat1 = np.asarray(feat1, np.float32)
    Ws = [np.asarray(w, np.float32) for w in (W00, W01, W10, W11)]
    bs = [np.asarray(b, np.float32) for b in (b00, b01, b10, b11)]

    nc = _get_graph()
    in_maps = []
    for c in range(NCORES):
        m, a0 = c // 2, (c % 2) * A
        in_maps.append(_prep_core(image, vectors, feat0, feat1, Ws, bs, m, a0))

    global _LAST_IN_MAPS
    _LAST_IN_MAPS = in_maps
    res = run_bass_kernel_spmd(nc, in_maps, core_ids=list(range(NCORES)))
    _LAST_RESULTS = res

    out1 = np.empty((B, N, F, 1), np.float32)
    out2 = np.empty((B, N, F, 3), np.float32)
    out3 = np.empty((B, N, F, 3), np.float32)
    out4 = np.empty((B, N, F, 1), np.float32)
    out5 = np.empty((B, N, F, 3), np.float32)
    for c in range(NCORES):
        m, a0 = c // 2, (c % 2) * A
        r = np.asarray(res.results[c]["out"], np.float32)  # (176, A)
        sl = slice(a0, a0 + A)
        out1[m, sl, :, 0] = r[0:16].T
        out3[m, sl] = r[16:64].reshape(F, 3, A).transpose(2, 0, 1)
        for j in range(3):
            out2[m, sl, :, j] = r[64 + 16 * j : 80 + 16 * j].T
        out4[m, sl, :, 0] = r[112:128].T
        for i in range(3):
            out5[m, sl, :, i] = r[128 + 16 * i : 144 + 16 * i].T
    return out1, out2, out3, out4, out5
